# revision 1
# baseline (speedup 1.0000x reference)
"""Trainium2 Bass kernel for nn_FFDGenerator: cubic B-spline free-form deformation.

Computes flow[b,c,x,y,z] = sum_{i,j,k} Wx[x,i]*Wy[y,j]*Wz[z,k]*mesh[b,c,i,j,k]
where Wx/Wy/Wz are dense per-axis cubic B-spline weight matrices (4 nonzeros
per row, spacing 8), mesh is (4,3,23,27,23), flow is (4,3,160,192,160).

Sharding: output x-axis split into 8 chunks of 20, one per NeuronCore.
Control-point mesh is replicated (bc-triples spread over partition bands).

Per-core pipeline (all contractions on the tensor engine):
  MM1: contract i -> A[xl, (bc,j,k)]       col-tiled over 4 partition bands
  T:   DVE 32x32 block transpose -> A_t[k, (bc,xl,j)] per band
  MM2: contract k -> C[(xl4,j32), (g,z)]   row-banded, data-stationary
  MM3: contract j -> out[(xl4,y32), z]     block-diag Wy weights, M=128
  Epilogue: PSUM -> SBUF copy with 1/27 scale, DMA to DRAM.

Numerics: matmuls run in fp16 at full speed using an exactness trick:
3*W has entries n/1024 with |n| <= 2048, exactly representable in fp16; the
data side is split hi/lo into two fp16 matmuls accumulating in fp32 PSUM.
The 27x scale is removed in the epilogue copy. End-to-end rel err ~1e-7.
"""

import numpy as np

import concourse.bass as bass
import concourse.mybir as mybir
from concourse.tile import TileContext
from concourse.bass_utils import run_bass_kernel_spmd

F16 = mybir.dt.float16
F32 = mybir.dt.float32

NCORES = 8
B, C = 4, 3
BC = B * C                    # 12 bc slices
X, Y, Z = 160, 192, 160
XL = X // NCORES              # 20 x per core
CX, CY, CZ = 23, 27, 23       # control points per axis
J32 = 32                      # padded j
K32 = 32                      # padded k
NB = 4                        # partition bands
BAND_BC = 3                   # bc per band
FREE1 = BAND_BC * J32 * K32   # 3072: per-band free size of meshT/A
NGRP = BC * XL // 4           # 60 groups of 4 (bc,x) slices
NYC = Y // 32                 # 6 y-chunks
GG = 3                        # groups per MM3 matmul
NGP = NGRP // GG              # 30 group-pairs
CHUNK = 512                   # MM1 free chunk (one PSUM bank of fp32)
NCH = FREE1 // CHUNK          # 6 chunks

# Merge each hi/lo matmul pair into ONE instruction: rhs = [hi | lo] along the
# free dim, out AP gets a stride-0 dim so the lo half accumulates into the
# same PSUM addresses (has_written semantics). Halves LDWEIGHTS+issue count.
# Note: CoreSim models the stride-0 write as overwrite, so sim numerics are
# only valid with MERGE_HILO=False; hardware accumulates correctly.
MERGE_HILO = False

_cache = {}


def _axis_weights3(n, sp, ncp):
    """Dense [n, ncp] matrix of 3x the cubic B-spline weights.

    With integer coordinates and spacing 8, 3*w = m/1024 with |m| <= 2048,
    exactly representable in fp16.
    """
    v = np.arange(n, dtype=np.float64) / sp
    f = np.floor(v)
    d = v - f
    w = np.stack(
        [
            (1 - d) ** 3 / 6,
            d**3 / 2 - d**2 + 2.0 / 3,
            -(d**3) / 2 + d**2 / 2 + d / 2 + 1.0 / 6,
            d**3 / 6,
        ],
        -1,
    )
    W3 = np.zeros((n, ncp))
    idx = f.astype(int)
    for a in range(4):
        W3[np.arange(n), idx + a] = np.round(3 * w[:, a] * 1024) / 1024
    assert np.all(np.float64(np.float16(W3)) == W3)
    return W3


def _host_weights():
    if "w" in _cache:
        return _cache["w"]
    W3x = _axis_weights3(X, 8, CX)
    W3y = _axis_weights3(Y, 8, CY)
    W3z = _axis_weights3(Z, 8, CZ)

    # wx3[core]: [128, 32] fp16, wxT chunk replicated on 4 partition bands
    wx3 = np.zeros((NCORES, 128, 32), np.float16)
    for core in range(NCORES):
        blk = W3x[core * XL : (core + 1) * XL, :].T.astype(np.float16)  # [23, 20]
        for q in range(NB):
            wx3[core, 32 * q : 32 * q + CX, :XL] = blk

    # wz3: [128, Z] fp16, wzT replicated on 4 partition bands
    wz3 = np.zeros((128, Z), np.float16)
    for q in range(NB):
        wz3[32 * q : 32 * q + CZ, :] = W3z.T.astype(np.float16)

    # wyb: [128, NYC*128] fp16 block-diag. Variant c covers the strided y-set
    # y = 6*yi + c (yi = 0..31): wyb[32b+j, 128c + 32b+yi] = W3y[6yi+c, j].
    # The stride-6 y interleave makes each slice's (yi, c, z) staging layout
    # land contiguously in flow[y, z], so one DMA stores a whole slice quad.
    wyb = np.zeros((128, NYC * 128), np.float16)
    for c in range(NYC):
        for b in range(4):
            for j in range(CY):
                wyb[32 * b + j, 128 * c + 32 * b : 128 * c + 32 * b + 32] = W3y[
                    c : c + 6 * 32 : 6, j
                ].astype(np.float16)

    _cache["w"] = (wx3, wz3, wyb)
    return _cache["w"]


def _prep_mesh(mesh):
    """mesh [4,3,23,27,23] f32 -> meshT4 hi/lo [128, FREE1] fp16.

    Partition 32q+i holds mesh[bc=3q+bcq, i, j, k] at free index
    bcq*J32*K32 + j*K32 + k (j, k zero-padded to 32).
    """
    m = np.asarray(mesh, np.float32).reshape(BC, CX, CY, CZ)
    mt = np.zeros((128, BAND_BC, J32, K32), np.float32)
    for q in range(NB):
        for bcq in range(BAND_BC):
            bc = 3 * q + bcq
            mt[32 * q : 32 * q + CX, bcq, :CY, :CZ] = m[bc]
    hi = mt.astype(np.float16)
    lo = (mt - hi.astype(np.float32)).astype(np.float16)
    return hi.reshape(128, FREE1), lo.reshape(128, FREE1)


def _build_program():
    if "nc" in _cache:
        return _cache["nc"]
    nc = bass.Bass()
    mesh_hi = nc.declare_dram_parameter("mesh_hi", [128, FREE1], F16, isOutput=False)
    mesh_lo = nc.declare_dram_parameter("mesh_lo", [128, FREE1], F16, isOutput=False)
    wx3 = nc.declare_dram_parameter("wx3", [128, 32], F16, isOutput=False)
    wz3 = nc.declare_dram_parameter("wz3", [128, Z], F16, isOutput=False)
    wyb = nc.declare_dram_parameter("wyb", [128, NYC * 128], F16, isOutput=False)
    flow = nc.declare_dram_parameter("flow", [BC * XL, Y, Z], F32, isOutput=True)

    # Store view: slice s = 12*gp + 4*gg + b, y = 6*yi + c. With staging laid
    # out [(b,yi) partitions, (c,z) free], a whole slice quad is one DMA:
    # dst offset = 960*(32b+yi) + 160c + z  (elements, relative to quad base).
    flowV = flow[:, :, :].rearrange(
        "(gp gg b) (yi c) z -> gp gg (b yi) c z", gg=GG, b=4, c=NYC
    )

    with TileContext(nc) as tc:
        with (
            tc.tile_pool(name="const", bufs=1) as cpool,
            tc.tile_pool(name="abuf", bufs=1) as apool,
            tc.tile_pool(name="cbuf", bufs=4) as cbpool,
            tc.tile_pool(name="stage", bufs=5) as spool,
            tc.tile_pool(name="ps1", bufs=1, space="PSUM") as ps1pool,
            tc.tile_pool(name="ps2", bufs=2, space="PSUM") as ps2pool,
            tc.tile_pool(name="ps3", bufs=2, space="PSUM") as ps3pool,
        ):
            # Per-chunk mesh tiles so MM1 chunk ch starts as soon as its own
            # slice of the mesh has landed (Tile deps are whole-tile). Each
            # chunk tile holds [hi | lo] halves side by side.
            wx = cpool.tile([128, 32], F16, tag="wx")
            nc.gpsimd.dma_start(out=wx[:, :], in_=wx3[:, :])
            m2c = []
            for ch in range(NCH):
                s = slice(ch * CHUNK, (ch + 1) * CHUNK)
                t2 = cpool.tile([128, 2 * CHUNK], F16, name=f"m2{ch}", tag=f"m2{ch}")
                nc.sync.dma_start(out=t2[:, :CHUNK], in_=mesh_hi[:, s])
                nc.gpsimd.dma_start(out=t2[:, CHUNK:], in_=mesh_lo[:, s])
                m2c.append(t2)
            wz = cpool.tile([128, Z], F16, tag="wz")
            wy = cpool.tile([128, NYC * 128], F16, tag="wy")
            nc.gpsimd.dma_start(out=wz[:, :], in_=wz3[:, :])
            nc.gpsimd.dma_start(out=wy[:, :], in_=wyb[:, :])

            def step0(ap, n):
                """Out-AP with a stride-0 dim: columns n..2n-1 accumulate
                onto columns 0..n-1 (PSUM has_written)."""
                return bass.AP(ap.tensor, ap.offset, [ap.ap[0], [0, 2], [1, n]])

            # ---- MM1 (contract i) + 32x32 block transpose ----
            # Separate A tiles per bc-triple (bcq) so the hi/lo split of one
            # triple (and the MM2 groups that consume it) can start while
            # later chunks are still in MM1.
            PB = J32 * K32  # 1024: per-bcq free size
            at = [apool.tile([128, PB], F32, name=f"at{b}", tag=f"at{b}")
                  for b in range(BAND_BC)]
            # hi/lo fp16 splits, stored permuted (j,x)->(x,j) so each MM2
            # lhsT is one contiguous 128-wide run (walrus: 1 free dim).
            ah = [apool.tile([128, PB], F16, name=f"ah{b}", tag=f"ah{b}")
                  for b in range(BAND_BC)]
            al = [apool.tile([128, PB], F16, name=f"al{b}", tag=f"al{b}")
                  for b in range(BAND_BC)]
            for ch in range(NCH):
                off = ch * CHUNK
                p1 = ps1pool.tile([128, CHUNK], F32, tag="p1")
                for q in range(NB):
                    band = slice(32 * q, 32 * q + CX)
                    if MERGE_HILO:
                        nc.tensor.matmul(
                            step0(p1[32 * q : 32 * q + 32, :], CHUNK),
                            lhsT=wx[band, :],
                            rhs=m2c[ch][band, :],
                            start=True,
                            stop=True,
                            tile_position=(32 * q, 32 * q),
                        )
                    else:
                        for t in range(2):
                            ms = slice(t * CHUNK, (t + 1) * CHUNK)
                            nc.tensor.matmul(
                                p1[32 * q : 32 * q + 32, :],
                                lhsT=wx[band, :],
                                rhs=m2c[ch][band, ms],
                                start=(t == 0),
                                stop=(t == 1),
                                tile_position=(32 * q, 32 * q),
                            )
                bq, half = ch // 2, (ch % 2) * CHUNK
                nc.vector.transpose(
                    out=at[bq][:, half : half + CHUNK], in_=p1[:, :]
                )
                if ch % 2 == 1:
                    b = bq
                    atP = at[b][:, :].rearrange(
                        "p (j x) -> p x j", j=J32, x=K32
                    )
                    ahV = ah[b][:, :].rearrange(
                        "p (x j) -> p x j", x=K32, j=J32
                    )
                    alV = al[b][:, :].rearrange(
                        "p (x j) -> p x j", x=K32, j=J32
                    )
                    nc.scalar.copy(out=ahV, in_=atP)
                    nc.vector.tensor_sub(out=alV, in0=atP, in1=ahV)


            # ---- MM2 (contract k) + C split + MM3 (contract j) + store ----
            # Fully interleaved per gp (= one supertile of 3 slice quads):
            # MM2 produces C hi/lo for this gp, MM3 consumes it immediately;
            # the staging tile is stored with 3 DMAs (one contiguous 492KB
            # block of flow per slice quad).
            inv27 = float(1.0 / 27.0)

            def emit_mm2(gp):
                """Produce C hi|lo (fp16) for supertile gp."""
                p2 = ps2pool.tile([128, 3 * Z], F32, tag="p2", name="p2")
                for sub in range(3):
                    g = gp * 3 + sub
                    bc = g // 5
                    q, bq, xg = bc // 3, bc % 3, g % 5
                    lo = 128 * xg
                    for t, aT in ((0, ah[bq]), (1, al[bq])):
                        nc.tensor.matmul(
                            p2[:, sub * Z : (sub + 1) * Z],
                            lhsT=aT[32 * q : 32 * q + CZ, lo : lo + 128],
                            rhs=wz[32 * q : 32 * q + CZ, :],
                            start=(t == 0),
                            stop=(t == 1),
                            tile_position=(32 * q, 0),
                        )
                chl = cbpool.tile([128, 2 * GG * Z], F16, name="chl", tag="chl")
                nc.scalar.copy(out=chl[:, : GG * Z], in_=p2[:, :])
                nc.vector.tensor_sub(
                    out=chl[:, GG * Z :], in0=p2[:, :], in1=chl[:, : GG * Z]
                )
                return chl

            def emit_mm3(gp, chl):
                """Contract j for supertile gp, scale, and store."""
                stg = spool.tile([128, NYC * GG * Z], F32, tag="stg", name="stg")
                stgV = stg[:, :].rearrange("p (c gg z) -> p gg c z", c=NYC, gg=GG)
                for cp in range(NYC // 2):  # pairs of y-variants
                    # two bank-aligned [128,480] halves in a 2-bank tile
                    p3 = ps3pool.tile([128, 1024], F32, tag="p3", name="p3")
                    for cc in range(2):
                        c = 2 * cp + cc
                        wslice = wy[:, 128 * c : 128 * (c + 1)]
                        po = p3[:, cc * 512 : cc * 512 + GG * Z]
                        for t in range(2):
                            cs2 = slice(t * GG * Z, (t + 1) * GG * Z)
                            nc.tensor.matmul(
                                po,
                                lhsT=wslice,
                                rhs=chl[:, cs2],
                                start=(t == 0),
                                stop=(t == 1),
                            )
                    # one strided copy moves both halves (skips bank padding)
                    src = p3[:, :].rearrange("p (a b) -> p a b", a=2, b=512)[
                        :, :, : GG * Z
                    ]
                    cs = slice(2 * cp * GG * Z, (2 * cp + 2) * GG * Z)
                    dst = stg[:, cs].rearrange("p (a b) -> p a b", a=2, b=GG * Z)
                    if cp % 2 == 0:
                        nc.scalar.mul(dst, src, inv27)
                    else:
                        nc.vector.tensor_scalar_mul(dst, src, inv27)
                for gg in range(GG):
                    eng = nc.gpsimd if gg == 1 else nc.sync
                    eng.dma_start(out=flowV[gp, gg], in_=stgV[:, gg])

            # Software pipeline (depth 2): MM3 for gp-2 is emitted after MM2
            # for gp, so C production runs well ahead of its consumption.
            pend = []
            for gp in range(NGP):
                pend.append((gp, emit_mm2(gp)))
                if len(pend) > 2:
                    emit_mm3(*pend.pop(0))
            for item in pend:
                emit_mm3(*item)

    # Walrus allows at most one sync-wait per matmul; split extras into
    # EventSemaphore instructions (same pass Bacc.compile runs).
    import bass_rust as _bass_rust

    _bass_rust.move_matmul_waits_to_ldweights(nc.m)
    _bass_rust.generate_event_semaphores(nc)

    _cache["nc"] = nc
    return nc


def _in_maps(mesh):
    wx3, wz3, wyb = _host_weights()
    mh, ml = _prep_mesh(mesh)
    return [
        {"mesh_hi": mh, "mesh_lo": ml, "wx3": wx3[core], "wz3": wz3, "wyb": wyb}
        for core in range(NCORES)
    ]


def kernel(mesh: np.ndarray) -> np.ndarray:
    nc = _build_program()
    in_maps = _in_maps(mesh)
    last_err = None
    for attempt in range(3):
        try:
            res = run_bass_kernel_spmd(nc, in_maps, list(range(NCORES))).results
            break
        except Exception as e:  # transient device wedge: retry
            last_err = e
    else:
        raise last_err
    full = np.empty((BC, X, Y, Z), np.float32)
    for core in range(NCORES):
        full[:, core * XL : (core + 1) * XL] = res[core]["flow"].reshape(
            BC, XL, Y, Z
        )
    return full.reshape(B, C, X, Y, Z)

